# revision 33
# baseline (speedup 1.0000x reference)
"""Trainium2 Bass kernel for CellMessageLayer (GNN message passing).

Strategy (8 NeuronCores, batch B=4):
  - 2 cores per batch element; each core owns half the destination nodes
    (2048) and processes exactly the edges whose dst falls in its half.
  - Host prep: sort edges by dst, group them into 16 chunks of 128 dst
    nodes per core, pad each chunk to a fixed capacity CAP (multiple of
    128). Padding edges use src=0 / dst_rel=-1 so they contribute nothing.
  - Device (per core, all bf16 matmuls with fp32 PSUM accumulation):
      gather src rows via transposed dma_gather (feature-major),
      edge MLP (L1 320x256, relu+bias, L2 256x256),
      scatter-mean via one-hot matmul accumulation in PSUM per 128-node
      chunk (one-hot built on-chip with iota + is_equal),
      PE-transpose of the aggregate into feature-major u_in,
      node MLP (512x256 relu, 256x256), residual + LayerNorm in fp32.
  - gamma/beta applied on host (exact; they are per-feature affine).
"""

import numpy as np
import ml_dtypes

import concourse.bacc as bacc
import concourse.mybir as mybir
import concourse.tile as tile
from concourse.bass import AP
from concourse.bass_utils import run_bass_kernel_spmd
from concourse.masks import make_identity

B, N, H, F, E = 4, 4096, 256, 64, 65536
P = 128
HALF = N // 2              # nodes per core
NCHUNK = HALF // P         # 16 chunks of 128 nodes per core
EPS = 1e-5
BF16 = mybir.dt.bfloat16
F32 = mybir.dt.float32
I16 = mybir.dt.int16
AF = mybir.ActivationFunctionType
OP = mybir.AluOpType


def _prep_inputs(cell_x, edge_index, edge_attr, mW1, mb1, mW2, mb2,
                 uW1, ub1, uW2, ub2):
    """Host-side sharding/prep. Returns (in_maps, CAP)."""
    bf = ml_dtypes.bfloat16
    src = np.clip(edge_index[..., 0], 0, N - 1).astype(np.int64)
    dst = np.clip(edge_index[..., 1], 0, N - 1).astype(np.int64)

    # fold mb2 into ub1 (mb2 passes through the mean aggregation)
    ub1p = (ub1 + uW1[H:2 * H].T @ mb2).astype(np.float32)

    per_b = []
    max_cnt = 0
    for b in range(B):
        order = np.argsort(dst[b], kind="stable")
        ds = dst[b][order]
        ss = src[b][order]
        chunk_of = ds >> 7                       # 0..31 global chunks
        counts = np.bincount(chunk_of, minlength=2 * NCHUNK)
        max_cnt = max(max_cnt, int(counts.max()))
        per_b.append((order, ds, ss, counts))
    CAP = max(128, -(-max_cnt // 128) * 128)     # per-chunk capacity
    ECAP = NCHUNK * CAP

    # shared weight tensors (bf16); W1b duplicated across both row halves
    wshared = {
        "w1": mW1.astype(bf),
        "w1bd": np.ascontiguousarray(
            np.concatenate([mW1[H:], mW1[H:]], axis=0).astype(bf)), "w2": mW2.astype(bf),
        "uw1": uW1.astype(bf), "uw2": uW2.astype(bf),
        "mb1w": np.ascontiguousarray(mb1.reshape(2, P).T.astype(np.float32)),
        "ub1w": np.ascontiguousarray(ub1p.reshape(2, P).T.astype(np.float32)),
    }

    in_maps = []
    for b in range(B):
        order, ds, ss, counts = per_b[b]
        node_cnt = np.bincount(ds, minlength=N).astype(np.float32)
        boundaries = np.cumsum(counts)
        starts = np.concatenate([[0], boundaries[:-1]])
        for half in range(2):
            src_pad = np.zeros(ECAP, np.int64)
            # node order rolled so this core's half sits at columns 0..HALF-1
            cxTf = np.ascontiguousarray(
                np.roll(cell_x[b].T, -half * HALF, axis=1).astype(bf))
            drel_pad = np.full(ECAP, -1.0, np.float32)
            ea_pad = np.zeros((ECAP, F), np.float32)
            for c in range(NCHUNK):
                g = half * NCHUNK + c
                s0, cnt = starts[g], counts[g]
                o = c * CAP
                src_pad[o:o + cnt] = (ss[s0:s0 + cnt] - half * HALF) % N
                drel_pad[o:o + cnt] = (ds[s0:s0 + cnt] - g * P).astype(np.float32)
                ea_pad[o:o + cnt] = edge_attr[b][order[s0:s0 + cnt]]
            # dma_gather wrapped index layout: idx i at [i%16, i//16], x8 groups
            idx_w = np.empty((P, ECAP // 16), np.int16)
            wrapped = src_pad.astype(np.int16).reshape(ECAP // 16, 16).T  # [16, S]
            for gr in range(8):
                idx_w[gr * 16:(gr + 1) * 16] = wrapped
            x = ea_pad.reshape(ECAP // 1024, 2, 512, F)
            eaP = np.ascontiguousarray(
                x.transpose(1, 3, 0, 2).reshape(2 * F, ECAP // 2).astype(bf))
            drel = np.ascontiguousarray(
                drel_pad.reshape(ECAP // P, P).T.astype(bf))
            cnt_half = node_cnt[half * HALF:(half + 1) * HALF]
            rcp = (1.0 / np.maximum(cnt_half, 1.0)).reshape(NCHUNK, P).T
            cxh = cell_x[b, half * HALF:(half + 1) * HALF].astype(np.float32)
            in_maps.append({
                "cxTf": cxTf,
                "idx": idx_w,
                "drel": drel,
                "eaT": eaP,
                "rcp": np.ascontiguousarray(rcp.astype(np.float32)),
                "cellxu": np.ascontiguousarray(cxh + ub2[None, :].astype(np.float32)),
                **wshared,
            })
    return in_maps, CAP


def build_program(CAP):
    """Build the per-core Bass/Tile program (same NEFF for all 8 cores)."""
    ECAP = NCHUNK * CAP
    NBLK = ECAP // 512
    TPC = CAP // P                     # 128-edge tiles per chunk

    nc = bacc.Bacc("TRN2", num_devices=1)
    d = {}
    d["cxTf"] = nc.dram_tensor("cxTf", [H, N], BF16, kind="ExternalInput")
    d["idx"] = nc.dram_tensor("idx", [P, ECAP // 16], I16, kind="ExternalInput")
    d["drel"] = nc.dram_tensor("drel", [P, ECAP // P], BF16, kind="ExternalInput")
    d["eaT"] = nc.dram_tensor("eaT", [2 * F, ECAP // 2], BF16, kind="ExternalInput")
    d["w1bd"] = nc.dram_tensor("w1bd", [P, H], BF16, kind="ExternalInput")
    d["rcp"] = nc.dram_tensor("rcp", [P, NCHUNK], F32, kind="ExternalInput")
    d["cellxu"] = nc.dram_tensor("cellxu", [HALF, H], F32, kind="ExternalInput")
    d["w1"] = nc.dram_tensor("w1", [H + F, H], BF16, kind="ExternalInput")
    d["w2"] = nc.dram_tensor("w2", [H, H], BF16, kind="ExternalInput")
    d["uw1"] = nc.dram_tensor("uw1", [2 * H, H], BF16, kind="ExternalInput")
    d["uw2"] = nc.dram_tensor("uw2", [H, H], BF16, kind="ExternalInput")
    d["mb1w"] = nc.dram_tensor("mb1w", [P, 2], F32, kind="ExternalInput")
    d["ub1w"] = nc.dram_tensor("ub1w", [P, 2], F32, kind="ExternalInput")
    d_out = nc.dram_tensor("out", [HALF, H], F32, kind="ExternalOutput")

    with tile.TileContext(nc) as tc:
        with (
            tc.tile_pool(name="const", bufs=1) as cp,
            tc.tile_pool(name="gx", bufs=8) as gxp,
            tc.tile_pool(name="ea", bufs=5) as eap,
            tc.tile_pool(name="r1", bufs=8) as r1p,
            tc.tile_pool(name="msg", bufs=8) as msgp,
            tc.tile_pool(name="s4", bufs=8) as s4p,
            tc.tile_pool(name="am", bufs=3) as amp,
            tc.tile_pool(name="ucx", bufs=3) as ucxp,
            tc.tile_pool(name="uln", bufs=3) as ulnp,
            tc.tile_pool(name="sc", bufs=4) as scp,
            tc.tile_pool(name="yst", bufs=4) as ystp,
            tc.tile_pool(name="dram", bufs=1, space="DRAM") as dramp,
            tc.tile_pool(name="psA", bufs=4, space="PSUM") as psA,
            tc.tile_pool(name="psB", bufs=3, space="PSUM") as psB,
            tc.tile_pool(name="psC", bufs=1, space="PSUM") as psC,
        ):
            # ---- constants (w1 on scalar queue + cxTf on sync queue first:
            # the Y table gates everything)
            # W1a (rows 0..255) as one [128, 2, 256] tile: one DMA on the
            # startup-critical path; W1b comes via w1bd (row-paired).
            w01 = cp.tile([P, 2, H], BF16, name="w01")
            nc.scalar.dma_start(
                out=w01[:],
                in_=d["w1"].ap()[0:2 * P, :].rearrange("(k p) h -> p k h", p=P))
            w1_sb = [w01[:, 0, :], w01[:, 1, :]]
            cxI = cp.tile([P, 2, N], BF16, name="cxI")
            Q = N // 4
            for q in range(4):
                nc.sync.dma_start(
                    out=cxI[:, :, q * Q:(q + 1) * Q],
                    in_=d["cxTf"].ap().rearrange("(k p) n -> p k n", p=P)
                    [:, :, q * Q:(q + 1) * Q])
            cxTf_sb = [cxI[:, k, :] for k in range(2)]
            rcp_sb = cp.tile([P, NCHUNK], F32)
            nc.scalar.dma_start(out=rcp_sb[:], in_=d["rcp"].ap())
            mb1_sb = cp.tile([P, 2], F32)
            nc.scalar.dma_start(out=mb1_sb[:], in_=d["mb1w"].ap())
            ub1_sb = cp.tile([P, 2], F32)
            nc.scalar.dma_start(out=ub1_sb[:], in_=d["ub1w"].ap())
            w2_sb = []
            for k in range(2):
                t = cp.tile([P, H], BF16, tag=f"w2_{k}")
                nc.sync.dma_start(out=t[:], in_=d["w2"].ap()[k * P:(k + 1) * P, :])
                w2_sb.append(t)
            uw1_sb = []
            for k in range(4):
                t = cp.tile([P, H], BF16, tag=f"uw1_{k}")
                nc.scalar.dma_start(out=t[:], in_=d["uw1"].ap()[k * P:(k + 1) * P, :])
                uw1_sb.append(t)
            uw2_sb = []
            for k in range(2):
                t = cp.tile([P, H], BF16, tag=f"uw2_{k}")
                nc.scalar.dma_start(out=t[:], in_=d["uw2"].ap()[k * P:(k + 1) * P, :])
                uw2_sb.append(t)
            iota4 = cp.tile([P, 4, P], BF16)
            nc.gpsimd.iota(iota4[:], pattern=[[0, 4], [1, P]], base=0,
                           channel_multiplier=0,
                           allow_small_or_imprecise_dtypes=True)
            ident = cp.tile([P, P], BF16)
            make_identity(nc, ident[:])
            eps_sb = cp.tile([P, 1], F32)
            nc.vector.memset(eps_sb[:], EPS)
            # u_in feature-major [512, HALF]; rows 0..255 alias cxTf tiles
            uin = [None, None] + [
                cp.tile([P, HALF], BF16, tag=f"uin_{k}", name=f"uin_{k}")
                for k in (2, 3)]
            uin[0], uin[1] = None, None  # cxI slices used directly

            # ---- Y table: ytbl[node] = (cell_x @ W1a)[node] (node-major bf16)
            ytbl = dramp.tile([N, H], BF16, name="ytbl")
            for g8 in range(N // (8 * P)):
                yst = ystp.tile([P, 8, H], BF16, tag="yst", name=f"yst_{g8}")
                for t8 in range(0, 8, 2):
                    psy = psB.tile([P, 2 * H], F32, space="PSUM", tag="ps2",
                                   name=f"psy_{g8}_{t8}")
                    for half in range(2):
                        yt = g8 * 8 + t8 + half
                        for k in range(2):
                            nc.tensor.matmul(
                                psy[:, half * H:(half + 1) * H],
                                lhsT=cxI[:, k, yt * P:(yt + 1) * P],
                                rhs=w1_sb[k], start=(k == 0), stop=(k == 1))
                    if t8 % 4 == 0:
                        nc.vector.tensor_copy(yst[:, t8:t8 + 2, :], psy[:])
                    else:
                        nc.scalar.copy(yst[:, t8:t8 + 2, :], psy[:])
                dst = ytbl[g8 * 8 * P:(g8 + 1) * 8 * P, :].rearrange(
                    "(t p) h -> p t h", p=P)
                nc.sync.dma_start(out=dst, in_=yst[:])

            idx_sb = cp.tile([P, ECAP // 16], I16)
            nc.sync.dma_start(out=idx_sb[:], in_=d["idx"].ap())
            drel_sb = cp.tile([P, ECAP // P], BF16)
            nc.sync.dma_start(out=drel_sb[:], in_=d["drel"].ap())
            w1bd_sb = cp.tile([P, H], BF16, name="w1bd_sb")
            nc.scalar.dma_start(out=w1bd_sb[:], in_=d["w1bd"].ap())

            # ---- edge phase (U-phase tiles interleaved) ----
            agg_live = {}
            agg_pair = {}
            pending_drain = []
            pending_u = __import__("collections").deque()
            chunks_done = 0

            am_live = {}

            def emit_mul(chunk):
                am = amp.tile([P, H], BF16, tag="am", name=f"am_{chunk}")
                nc.vector.tensor_scalar_mul(
                    am[:], agg_live[chunk][:], rcp_sb[:, chunk:chunk + 1])
                del agg_live[chunk]
                am_live[chunk] = am

            def emit_drain(chunk):
                am = am_live.pop(chunk)
                for hh in range(2):
                    tp = psB.tile([P, P], BF16, space="PSUM", tag="ps2",
                                  name=f"aggT_{chunk}_{hh}")
                    nc.tensor.transpose(tp[:], am[:, hh * P:(hh + 1) * P],
                                        ident[:])
                    nc.vector.tensor_copy(
                        uin[2 + hh][:, chunk * P:(chunk + 1) * P], tp[:])

            def emit_u1(nt, ru):
                for m in range(2):
                    psu = psA.tile([P, 512], F32, space="PSUM", tag="ps1",
                                   name=f"psu_{nt}_{m}")
                    for k in range(4):
                        rhs = (cxI[:, k, nt * 512:(nt + 1) * 512] if k < 2
                               else uin[k][:, nt * 512:(nt + 1) * 512])
                        nc.tensor.matmul(psu[:],
                                         lhsT=uw1_sb[k][:, m * P:(m + 1) * P],
                                         rhs=rhs,
                                         start=(k == 0), stop=(k == 3))
                    rt = r1p.tile([P, 512], BF16, tag=f"ru_{m}",
                                  name=f"ru_{nt}_{m}")
                    nc.scalar.activation(rt[:], psu[:], AF.Relu,
                                         bias=ub1_sb[:, m:m + 1])
                    ru.append(rt)

            def emit_u2(nt, ru, hh):
                if True:
                    psu2 = psB.tile([P, 512], F32, space="PSUM", tag="ps2",
                                    name=f"psu2_{nt}_{hh}")
                    for t2 in range(2):
                        t = hh * 2 + t2
                        for k in range(2):
                            nc.tensor.matmul(
                                psu2[:, t2 * H:(t2 + 1) * H],
                                lhsT=ru[k][:, t * P:(t + 1) * P],
                                rhs=uw2_sb[k][:], start=(k == 0), stop=(k == 1))
                    for t2 in range(2):
                        g = nt * 4 + hh * 2 + t2
                        cx = ucxp.tile([P, H], F32, tag="cx", name=f"cx_{g}")
                        nc.sync.dma_start(
                            out=cx[:], in_=d["cellxu"].ap()[g * P:(g + 1) * P, :])
                        hsb = ulnp.tile([P, H], F32, tag="h", name=f"h_{g}")
                        nc.vector.tensor_add(hsb[:], psu2[:, t2 * H:(t2 + 1) * H],
                                             cx[:])
                        red = scp.tile([P, 1], F32, tag="red", name=f"red_{g}")
                        nc.vector.tensor_reduce(red[:], hsb[:],
                                                axis=mybir.AxisListType.X,
                                                op=OP.add)
                        negmu = scp.tile([P, 1], F32, tag="negmu",
                                         name=f"negmu_{g}")
                        nc.vector.tensor_scalar_mul(negmu[:], red[:], -1.0 / H)
                        xc = ulnp.tile([P, H], F32, tag="xc", name=f"xc_{g}")
                        nc.vector.tensor_scalar_add(xc[:], hsb[:], negmu[:])
                        sq = ulnp.tile([P, H], F32, tag="sq", name=f"sq_{g}")
                        varsum = scp.tile([P, 1], F32, tag="var",
                                          name=f"var_{g}")
                        nc.scalar.activation(sq[:], xc[:], AF.Square,
                                             accum_out=varsum[:])
                        std = scp.tile([P, 1], F32, tag="std", name=f"std_{g}")
                        nc.scalar.activation(std[:], varsum[:], AF.Sqrt,
                                             scale=1.0 / H, bias=eps_sb[:])
                        rstd = scp.tile([P, 1], F32, tag="rstd",
                                        name=f"rstd_{g}")
                        nc.vector.reciprocal(rstd[:], std[:])
                        osb = ulnp.tile([P, H], F32, tag="o", name=f"o_{g}")
                        nc.scalar.activation(osb[:], xc[:], AF.Copy,
                                             scale=rstd[:])
                        nc.sync.dma_start(
                            out=d_out.ap()[g * P:(g + 1) * P, :], in_=osb[:])

            for blk in range(NBLK):
                gx = gxp.tile([P, 2, 512], BF16)
                nc.gpsimd.dma_gather(
                    out_ap=gx[:], in_ap=ytbl[:],
                    idxs_ap=idx_sb[:, blk * 32:(blk + 1) * 32],
                    num_idxs=512, num_idxs_reg=512, elem_size=H,
                    transpose=True)
                if blk % 2 == 0:
                    ea2 = eap.tile([2 * F, 512], BF16, tag="ea2",
                                   name=f"ea2_{blk}")
                    nc.sync.dma_start(
                        out=ea2[:],
                        in_=d["eaT"].ap()[:, (blk // 2) * 512:(blk // 2 + 1) * 512])
                # L1 psum = ea @ W1b, then += gathered Y rows (identity MM).
                # The two K=64 ea matmuls of a block pair run on disjoint PE
                # row groups (0-63 / 64-127) so they execute concurrently.
                if blk % 2 == 0:
                    pair_ps = []
                    for b2 in range(2):
                        ps_m = []
                        for m in range(2):
                            ps1 = psA.tile([P, 512], F32, space="PSUM",
                                           tag="ps1", name=f"ps1_{blk + b2}_{m}")
                            ps_m.append(ps1)
                        pair_ps.append(ps_m)
                    for m in range(2):
                        for b2 in range(2):
                            nc.tensor.matmul(
                                pair_ps[b2][m][:],
                                lhsT=w1bd_sb[b2 * F:(b2 + 1) * F,
                                             m * P:(m + 1) * P],
                                rhs=ea2[b2 * F:(b2 + 1) * F, :],
                                start=True, stop=False)
                ps1s = pair_ps[blk % 2]
                r1 = []
                for m in range(2):
                    nc.tensor.matmul(ps1s[m][:], lhsT=ident[:],
                                     rhs=gx[:, m, :], start=False, stop=True)
                for m in range(2):
                    rt = r1p.tile([P, 512], BF16, tag=f"r1_{m}",
                                  name=f"r1_{blk}_{m}")
                    nc.scalar.activation(rt[:], ps1s[m][:], AF.Relu,
                                         bias=mb1_sb[:, m:m + 1])
                    r1.append(rt)
                # deferred chunk drains; at most one U piece per block
                for _c in pending_drain:
                    emit_drain(_c)
                    chunks_done += 1
                    if chunks_done % 4 == 0:
                        nt = chunks_done // 4 - 1
                        ru = []
                        pending_u.append(lambda nt=nt, ru=ru: emit_u1(nt, ru))
                        pending_u.append(lambda nt=nt, ru=ru: emit_u2(nt, ru, 0))
                        pending_u.append(lambda nt=nt, ru=ru: emit_u2(nt, ru, 1))
                pending_drain = []
                if pending_u:
                    pending_u.popleft()()
                # L2: edge-major msg tiles, 2 per psum bank
                msgs = []
                for hh in range(2):
                    ps2 = psB.tile([P, 512], F32, space="PSUM", tag="ps2")
                    for t2 in range(2):
                        t = hh * 2 + t2
                        for k in range(2):
                            nc.tensor.matmul(
                                ps2[:, t2 * H:(t2 + 1) * H],
                                lhsT=r1[k][:, t * P:(t + 1) * P],
                                rhs=w2_sb[k][:], start=(k == 0), stop=(k == 1))
                    mt = msgp.tile([P, 512], BF16, tag=f"msg_{hh}")
                    if hh == 0:
                        nc.vector.tensor_copy(mt[:], ps2[:])
                    elif blk % 2 == 0:
                        nc.vector.tensor_copy(mt[:], ps2[:])
                    else:
                        nc.scalar.copy(mt[:], ps2[:])
                    msgs.append(mt)
                # one-hot S for the 4 edge tiles of this block
                s4 = s4p.tile([P, 4, P], BF16)
                drap = drel_sb[:, blk * 4:(blk + 1) * 4]
                dr_b = AP(drap.tensor, drap.offset,
                          [drap.ap[0], [drap.ap[1][0], 4], [0, P]])
                nc.vector.tensor_tensor(out=s4[:], in0=iota4[:], in1=dr_b,
                                        op=OP.is_equal)
                # scatter-accumulate into per-chunk PSUM
                for t in range(4):
                    g = blk * 4 + t
                    chunk, pos = g // TPC, g % TPC
                    if pos == 0:
                        if chunk % 2 == 0:
                            agg_pair[chunk // 2] = psC.tile(
                                [P, 2 * H], F32, space="PSUM", tag="agg",
                                name=f"aggp_{chunk // 2}")
                        agg_live[chunk] = agg_pair[chunk // 2][
                            :, (chunk % 2) * H:(chunk % 2 + 1) * H]
                    nc.tensor.matmul(agg_live[chunk][:], lhsT=s4[:, t, :],
                                     rhs=msgs[t // 2][:, (t % 2) * H:(t % 2 + 1) * H],
                                     start=(pos == 0), stop=(pos == TPC - 1))
                    if pos == TPC - 1:
                        emit_mul(chunk)
                        pending_drain.append(chunk)

            # ---- leftover drains / U pieces ----
            for _c in pending_drain:
                emit_drain(_c)
                chunks_done += 1
                if chunks_done % 4 == 0:
                    nt = chunks_done // 4 - 1
                    ru = []
                    pending_u.append(lambda nt=nt, ru=ru: emit_u1(nt, ru))
                    pending_u.append(lambda nt=nt, ru=ru: emit_u2(nt, ru, 0))
                    pending_u.append(lambda nt=nt, ru=ru: emit_u2(nt, ru, 1))
            while pending_u:
                pending_u.popleft()()
    nc.compile()
    return nc


_CACHE = {}


def kernel(cell_x, edge_index, edge_attr, mW1, mb1, mW2, mb2,
           uW1, ub1, uW2, ub2, gamma, beta):
    cell_x = np.asarray(cell_x, np.float32)
    edge_index = np.asarray(edge_index)
    edge_attr = np.asarray(edge_attr, np.float32)
    in_maps, CAP = _prep_inputs(cell_x, edge_index, edge_attr,
                                np.asarray(mW1, np.float32), np.asarray(mb1, np.float32),
                                np.asarray(mW2, np.float32), np.asarray(mb2, np.float32),
                                np.asarray(uW1, np.float32), np.asarray(ub1, np.float32),
                                np.asarray(uW2, np.float32), np.asarray(ub2, np.float32))
    if CAP not in _CACHE:
        _CACHE[CAP] = build_program(CAP)
    nc = _CACHE[CAP]
    res = run_bass_kernel_spmd(nc, in_maps, core_ids=list(range(8)))
    out = np.empty((B, N, H), np.float32)
    for core in range(8):
        b, half = core // 2, core % 2
        out[b, half * HALF:(half + 1) * HALF] = res.results[core]["out"]
    out = out * np.asarray(gamma, np.float32)[None, None, :] \
        + np.asarray(beta, np.float32)[None, None, :]
    return out
